# revision 18
# baseline (speedup 1.0000x reference)
"""DisplacementNet Trainium2 kernel: kNN graph + 4 GNN message-passing layers.

Sharding: data-parallel over nodes, 1024 rows per core (8 cores). Coordinates
and weights are replicated; the kNN candidate set is the full 8192 nodes. The
per-layer feature table is all-gathered across cores between layers.

Self-contained: hardcodes all shapes; no sibling imports.
"""
import numpy as np

import concourse.bass as bass
import concourse.bacc as bacc
import concourse.mybir as mybir
import concourse.tile as tile
from concourse import library_config
from concourse.tile_rust import add_dep_helper
from concourse.vector_clock import ScopedClock

N = 8192
NC = 8
NLOC = N // NC          # 1024
TPC = NLOC // 128       # 8 tiles of 128 nodes per core
K = 12
GW = 192
GL = 4
MIXW = 392              # 390 + ones + pad
LN_EPS = 1e-3
NEG_INF = -3.0e38
F16TAB = True
F32 = mybir.dt.float32
AF = mybir.ActivationFunctionType
ALU = mybir.AluOpType
AX = mybir.AxisListType

# ---------------------------------------------------------------- tile patch

_MAXW = 1


def _patched_drain_and_barrier(self, tick_clock, wait_clock):
    nc = self.nc
    drain_inst = nc.sync.drain()
    wait_clock.add_sem_waits(
        drain_inst.ins, ScopedClock({None: tick_clock.global_clock})
    )
    si = drain_inst.ins.sync_info
    waits = list(si.on_wait or []) if si is not None else []
    if len(waits) > _MAXW:
        drain_inst.ins.sync_info = mybir.SyncInfo(
            on_wait=waits[:_MAXW], on_update=list(si.on_update or [])
        )
        rest = waits[_MAXW:]
        for i in range(0, len(rest), _MAXW):
            d2 = nc.sync.drain()
            d2.ins.sync_info = mybir.SyncInfo(on_wait=rest[i : i + _MAXW], on_update=[])
    nc.all_engine_barrier()
    assert self.sems is not None
    popped = nc._tile_sem_poison_stack.pop()
    assert popped is self._sem_poison
    nc.clear_and_free_semaphores(list(self.sems.allocated().values()))
    nc.all_engine_barrier()


tile.TileContext._drain_and_barrier = _patched_drain_and_barrier


def _split_waits(nc):
    """This walrus build allows one sync-wait per instruction; hoist extras
    onto same-engine NOPs inserted just before."""
    for f in nc.m.functions:
        for bb in f.blocks:
            out, changed = [], False
            for ins in bb.instructions:
                si = ins.sync_info
                waits = list(si.on_wait or []) if si is not None else []
                if len(waits) > 1:
                    changed = True
                    for i, w in enumerate(waits[:-1]):
                        nop = mybir.InstNoOp(
                            name=f"{ins.name}_wsplit{i}", engine=ins.engine,
                            ins=[], outs=[],
                        )
                        nop.sync_info = mybir.SyncInfo(on_wait=[w], on_update=[])
                        out.append(nop)
                    ins.sync_info = mybir.SyncInfo(
                        on_wait=[waits[-1]], on_update=list(si.on_update or [])
                    )
                out.append(ins)
            if changed:
                bb.instructions = out


# ---------------------------------------------------------------- kernel body

_GQ = [0]


def gather_split(nc, out_tile, tab, widx_t, elem):
    """dma_gather with >1024 indices fails on HW; split into 1024 + 512.
    (Only SWDGE queue 0 exists on this build, so calls serialize there.)"""
    insts = []
    for (m0, cnt) in ((0, 1024), (1024, 512)):
        q = 0
        insts.append(nc.gpsimd.dma_gather(
            out_ap=out_tile[:, (m0 // 128) * elem:((m0 + cnt) // 128) * elem]
                .rearrange("p (j e) -> p j e", e=elem),
            in_ap=tab[:],
            idxs_ap=widx_t[:, m0 // 16:(m0 + cnt) // 16].bitcast(mybir.dt.int16),
            num_idxs=cnt, num_idxs_reg=cnt, elem_size=elem, queue_num=q))
    return insts


def build_kernel(n_cores=NC, split=True, phases="full"):
    nc = bacc.Bacc(None)
    nc.gpsimd.load_library(library_config.mlp)

    F16 = mybir.dt.float16
    F32R = mybir.dt.float32r
    xsq4r = nc.dram_tensor("xsq4r", [4, N], F32, kind="ExternalInput")
    lhs4r = nc.dram_tensor("lhs4r", [4, NLOC], F32, kind="ExternalInput")
    Wp16 = nc.dram_tensor("Wp16", [20, GW], F16, kind="ExternalInput")
    WgA = nc.dram_tensor("WgA", [128, 16 * GW], F32, kind="ExternalInput")
    xst16 = nc.dram_tensor("xst16", [128, 64 * 64], F16, kind="ExternalInput")
    xonn = nc.dram_tensor("xonn", [128, 3 * TPC], F32, kind="ExternalInput")
    feat16 = nc.dram_tensor("feat16", [20, N], F16, kind="ExternalInput")
    fown16 = nc.dram_tensor("fown16", [20, NLOC], F16, kind="ExternalInput")
    ident = nc.dram_tensor("ident", [128, 128], F32, kind="ExternalInput")
    Wo1 = nc.dram_tensor("Wo1", [128, 3], F32, kind="ExternalInput")
    Wo2 = nc.dram_tensor("Wo2", [65, 3], F32, kind="ExternalInput")
    y = nc.dram_tensor("y", [NLOC, 3], F32, kind="ExternalOutput")

    with tile.TileContext(nc) as tc:
        with tc.tile_pool(name="const", bufs=1) as cpool, \
             tc.tile_pool(name="big", bufs=1) as big, \
             tc.tile_pool(name="nd", bufs=2) as ndpool, \
             tc.tile_pool(name="gath", bufs=2) as gpool, \
             tc.tile_pool(name="small", bufs=2) as small, \
             tc.tile_pool(name="work", bufs=3) as work, \
             tc.tile_pool(name="tbp", bufs=2) as tbp, \
             tc.tile_pool(name="partp", bufs=8) as partp, \
             tc.tile_pool(name="g1p", bufs=1) as g1p, \
             tc.tile_pool(name="ndps", bufs=2, space="PSUM") as ndps, \
             tc.tile_pool(name="tps", bufs=2, space="PSUM") as tps, \
             tc.tile_pool(name="mmps", bufs=2, space="PSUM") as mmps, \
             tc.tile_pool(name="dram", bufs=1, space="DRAM") as dp:

            # ---- constants into SBUF
            feat_sb = cpool.tile([20, N], F16)
            nc.sync.dma_start(out=feat_sb[:], in_=feat16[:])
            lhs_r = cpool.tile([4, NLOC], F32)
            nc.sync.dma_start(out=lhs_r[:], in_=lhs4r[:])
            rhs_r = cpool.tile([4, N], F32)
            nc.sync.dma_start(out=rhs_r[:], in_=xsq4r[:])
            Wp_sb = cpool.tile([20, GW], F16)
            nc.sync.dma_start(out=Wp_sb[:], in_=Wp16[:])
            Wg_sb = cpool.tile([128, 16 * GW], F32)
            nc.sync.dma_start(out=Wg_sb[:], in_=WgA[:])
            xonn_sb = cpool.tile([128, 3 * TPC], F32)
            nc.sync.dma_start(out=xonn_sb[:], in_=xonn[:])
            fown_sb = cpool.tile([20, NLOC], F16)
            nc.sync.dma_start(out=fown_sb[:], in_=fown16[:])
            id_sb = cpool.tile([128, 128], F32)
            nc.sync.dma_start(out=id_sb[:], in_=ident[:])
            Wo1_sb = cpool.tile([128, 3], F32)
            nc.sync.dma_start(out=Wo1_sb[:], in_=Wo1[:])
            Wo2_sb = cpool.tile([65, 3], F32)
            nc.sync.dma_start(out=Wo2_sb[:], in_=Wo2[:])
            eps_sb = cpool.tile([128, 1], F32)
            nc.vector.memset(eps_sb[:], LN_EPS)
            zpad = cpool.tile([128, 512], mybir.dt.float16)
            nc.vector.memset(zpad[:], 0.0)
            goff32 = cpool.tile([128, 64], mybir.dt.int32)
            nc.gpsimd.iota(goff32[:], pattern=[[1024, 8], [0, 8]], base=0,
                           channel_multiplier=0)

            # ---- DRAM buffers
            TDT = mybir.dt.float16 if F16TAB else F32
            TE = 256 if F16TAB else GW
            tab0 = dp.tile([N, 256], F16)
            tabs = [tab0]
            for l in range(1, GL):
                tabs.append(dp.tile([N, TE], TDT, name=f"tab{l}"))
            ibs = [dp.tile([NLOC, TE], TDT, name=f"ib{l}") for l in range(GL - 1)]
            dwidxs = [dp.tile([16, 96], mybir.dt.uint16, name=f"dw{t}") for t in range(TPC)]

            # ---- persistent SBUF state
            widxs = [big.tile([128, 96], mybir.dt.uint16, name=f"wx{t}") for t in range(TPC)]
            mixbuf = big.tile([128, TPC * MIXW], F32)
            h4buf = big.tile([128, TPC * GW], F32)
            h16buf = big.tile([128, TPC * 256], mybir.dt.float16)
            nc.vector.memset(h16buf[:], 0.0)
            ybuf = big.tile([128, TPC * 3], F32)
            mix3 = mixbuf[:].rearrange("p (t c) -> p t c", c=MIXW)
            nc.vector.memset(mix3[:, :, 390:391], 1.0)
            nc.vector.memset(mix3[:, :, 391:392], 0.0)

            # ---- phase A: full hcur0 table (graph_proj), every core redundantly
            for G in range(8):
                tb = tbp.tile([128, 8 * 256], F16, tag="tb")
                tb3 = tb[:].rearrange("p (i e) -> p i e", e=256)
                # x / x^2 stats into cols 192:256 of each row (rest zeros)
                nc.sync.dma_start(out=tb3[:, :, 192:256],
                                  in_=xst16[:, 512 * G:512 * (G + 1)]
                                  .rearrange("p (i e) -> p i e", e=64))
                for i in range(8):
                    T = 8 * G + i
                    ps = mmps.tile([128, GW], F32, tag="mmps")
                    nc.tensor.matmul(ps[:], lhsT=feat_sb[:, 128 * T:128 * (T + 1)],
                                     rhs=Wp_sb[:], start=True, stop=True)
                    nc.scalar.copy(tb3[:, i, 0:GW], ps[:])
                # one batched DMA per 1024 rows: dst rows 128T+p, 512B descs
                dst = tab0[1024 * G:1024 * (G + 1), :].rearrange("(i p) e -> p i e", p=128)
                nc.scalar.dma_start(out=dst, in_=tb3)

            # own rows of hcur0 straight from per-core features (no gather)
            for t in range(TPC):
                ps = mmps.tile([128, GW], F32, tag="mmps")
                nc.tensor.matmul(ps[:], lhsT=fown_sb[:, 128 * t:128 * (t + 1)],
                                 rhs=Wp_sb[:], start=True, stop=True)
                nc.scalar.copy(mix3[:, t, 0:GW], ps[:])

            if phases == "A":
                for t in range(TPC):
                    yt = small.tile([128, 3], F32, tag="yt")
                    nc.vector.tensor_copy(yt[:], mix3[:, t, 0:3])
                    nc.sync.dma_start(out=y[128 * t:128 * (t + 1), :], in_=yt[:])
            # ---- phase B: kNN + layer-1 gather per tile
            for t in range(TPC if phases != "A" else 0):
                # per-1024-group distances into PSUM; top-8 values + local
                # indices scanned straight from PSUM (no SBUF staging), then a
                # packed-key merge: quantize value to the row's [16th, 1st]
                # range and embed the 13-bit global index in the low mantissa
                # bits, so the global top-12 needs no full-row max_index.
                cand = small.tile([128, 64], F32, tag="cand")
                cidx = small.tile([128, 64], mybir.dt.uint16, tag="cidx")
                for gq in range(8):
                    ps = ndps.tile([128, 1024], F32, tag="ndps")
                    for q in range(2):
                        nc.tensor.matmul(
                            ps[:, 512 * q:512 * (q + 1)],
                            lhsT=lhs_r[:, 128 * t:128 * (t + 1)],
                            rhs=rhs_r[:, 1024 * gq + 512 * q:1024 * gq + 512 * (q + 1)],
                            start=True, stop=True)
                    nc.vector.max(cand[:, 8 * gq:8 * (gq + 1)], ps[:])
                    nc.vector.max_index(cidx[:, 8 * gq:8 * (gq + 1)],
                                        cand[:, 8 * gq:8 * (gq + 1)], ps[:])
                v1 = small.tile([128, 8], F32, tag="v1")
                nc.vector.max(v1[:], cand[:])
                cand2 = small.tile([128, 64], F32, tag="cand2")
                nc.vector.match_replace(cand2[:], v1[:], cand[:], NEG_INF)
                v2 = small.tile([128, 8], F32, tag="v2")
                nc.vector.max(v2[:], cand2[:])
                rng = small.tile([128, 1], F32, tag="rng")
                nc.vector.tensor_sub(rng[:], v1[:, 0:1], v2[:, 7:8])
                rngc = small.tile([128, 1], F32, tag="rngc")
                nc.vector.tensor_scalar_max(rngc[:], rng[:], 1e-20)
                rcp = small.tile([128, 1], F32, tag="rcp")
                nc.vector.reciprocal(rcp[:], rngc[:])
                rcp2 = small.tile([128, 1], F32, tag="rcp2")
                nc.vector.tensor_scalar_mul(rcp2[:], rcp[:], 1048576.0)
                t0 = small.tile([128, 64], F32, tag="t0")
                nc.vector.tensor_scalar(t0[:], cand[:], v2[:, 7:8], rcp2[:],
                                        op0=ALU.subtract, op1=ALU.mult)
                t1 = small.tile([128, 64], F32, tag="t1")
                nc.vector.tensor_scalar(t1[:], t0[:], 0.0, 1048576.0,
                                        op0=ALU.max, op1=ALU.add)
                tq = small.tile([128, 64], mybir.dt.int32, tag="tq")
                nc.vector.tensor_scalar(tq[:], t1[:].bitcast(mybir.dt.int32),
                                        -8192, None, op0=ALU.bitwise_and)
                gidx = small.tile([128, 64], mybir.dt.int32, tag="gidx")
                nc.vector.tensor_copy(gidx[:], cidx[:])
                gidx2 = small.tile([128, 64], mybir.dt.int32, tag="gidx2")
                nc.vector.tensor_add(gidx2[:], gidx[:], goff32[:])
                packedi = small.tile([128, 64], mybir.dt.int32, tag="packedi")
                nc.vector.tensor_tensor(packedi[:], gidx2[:], tq[:],
                                        op=ALU.bitwise_or)
                kp1 = small.tile([128, 8], F32, tag="kp1")
                nc.vector.max(kp1[:], packedi[:].bitcast(F32))
                pk2 = small.tile([128, 64], F32, tag="pk2")
                nc.vector.match_replace(pk2[:], kp1[:], packedi[:].bitcast(F32),
                                        NEG_INF)
                kp2 = small.tile([128, 8], F32, tag="kp2")
                nc.vector.max(kp2[:], pk2[:])
                knp = small.tile([128, 12], mybir.dt.int32, tag="knp")
                nc.vector.tensor_scalar(knp[:, 0:7],
                                        kp1[:, 1:8].bitcast(mybir.dt.int32),
                                        8191, None, op0=ALU.bitwise_and)
                nc.vector.tensor_scalar(knp[:, 7:12],
                                        kp2[:, 0:5].bitcast(mybir.dt.int32),
                                        8191, None, op0=ALU.bitwise_and)
                # wrapped idx build through DRAM: m = j*128+p at [m%16, 96t + m//16]
                kn = small.tile([128, 12], mybir.dt.uint16, tag="kn")
                nc.vector.tensor_copy(kn[:], knp[:])
                d1 = dwidxs[t][:].rearrange("q (j phi) -> phi q j", phi=8)
                w1 = nc.sync.dma_start(out=d1, in_=kn[:])
                # replicate the 16-partition wrapped list to all 8 groups by doubling
                reps = []
                r = nc.sync.dma_start(out=widxs[t][0:16, :], in_=dwidxs[t][:])
                add_dep_helper(r.ins, w1.ins, True, "rep after build")
                reps.append(r)
                for (src_n, eng) in ((16, nc.scalar), (32, nc.sync), (64, nc.scalar)):
                    r = eng.dma_start(out=widxs[t][src_n:2 * src_n, :],
                                      in_=widxs[t][0:src_n, :])
                    reps.append(r)

                if phases == "BK":
                    yt = small.tile([128, 3], F32, tag="yt")
                    nc.vector.tensor_copy(yt[:], kn[:, 0:3])
                    nc.sync.dma_start(out=y[128 * t:128 * (t + 1), :], in_=yt[:])
                    continue
                # layer-1 feature gather; x/x2 stats ride along in cols 192:198
                # (SWDGE gather tops out at 1024 indices -> split 1024 + 512)
                g = g1p.tile([128, K * 256], F16, tag="g")
                gi = gather_split(nc, g, tab0, widxs[t], 256)
                if phases.startswith("BG2"):
                    yt = small.tile([128, 3], F32, tag="yt")
                    nc.vector.tensor_copy(yt[:], g[:, 0:3])
                    nc.sync.dma_start(out=y[128 * t:128 * (t + 1), :], in_=yt[:])
                    for r in reps:
                        for gg in gi:
                            add_dep_helper(gg.ins, r.ins, True, "gather after widx")
                    continue
                if phases == "BG":
                    gv = g[:].rearrange("p (j e) -> p j e", e=256)[:, :, 0:GW]
                    nc.vector.reduce_sum(mix3[:, t, GW:2 * GW],
                                   gv.rearrange("p j e -> p e j"), axis=AX.X)
                    yt = small.tile([128, 3], F32, tag="yt")
                    nc.vector.tensor_copy(yt[:], mix3[:, t, GW:GW + 3])
                    nc.sync.dma_start(out=y[128 * t:128 * (t + 1), :], in_=yt[:])
                    for r in reps:
                        for gg in gi:
                            add_dep_helper(gg.ins, r.ins, True, "gather after widx")
                    continue
                for r in reps:
                    for gg in gi:
                        add_dep_helper(gg.ins, r.ins, True, "gather after widx")

                # agg-sum for layer 1 via f16 tree-adds (2x DVE mode); the
                # x/x^2 stat columns (192:198) ride along in the same adds
                gv = g[:].rearrange("p (j e) -> p j e", e=256)
                t6 = gpool.tile([128, 6 * 256], F16, tag="t6")
                t6v = t6[:].rearrange("p (j e) -> p j e", e=256)
                nc.vector.tensor_add(t6v, gv[:, 0:6, :], gv[:, 6:12, :])
                t3 = gpool.tile([128, 3 * 256], F16, tag="t3")
                t3v = t3[:].rearrange("p (j e) -> p j e", e=256)
                nc.vector.tensor_add(t3v, t6v[:, 0:3, :], t6v[:, 3:6, :])
                t1 = gpool.tile([128, 256], F16, tag="t1")
                nc.vector.tensor_add(t1[:], t3[:, 0:256], t3[:, 256:512])
                nc.vector.tensor_add(mix3[:, t, GW:2 * GW], t1[:, 0:GW],
                                     t3[:, 512:512 + GW])
                # rel stats (layer-invariant): mean + population std via E[x2]-E[x]^2
                s6 = small.tile([128, 6], F32, tag="s6")
                nc.vector.tensor_add(s6[:], t1[:, GW:GW + 6],
                                     t3[:, 512 + GW:512 + GW + 6])
                m6 = small.tile([128, 6], F32, tag="m6")
                nc.vector.tensor_scalar_mul(m6[:], s6[:], 1.0 / K)
                nc.vector.tensor_sub(mix3[:, t, 384:387], m6[:, 0:3], xonn_sb[:, 3 * t:3 * (t + 1)])
                msq = small.tile([128, 3], F32, tag="msq")
                nc.vector.tensor_mul(msq[:], m6[:, 0:3], m6[:, 0:3])
                var3 = small.tile([128, 3], F32, tag="var3")
                nc.vector.tensor_sub(var3[:], m6[:, 3:6], msq[:])
                var3r = small.tile([128, 3], F32, tag="var3r")
                nc.vector.tensor_scalar_max(var3r[:], var3[:], 0.0)
                nc.scalar.activation(mix3[:, t, 387:390], var3r[:], AF.Sqrt)

            if phases == "B":
                for t in range(TPC):
                    yt = small.tile([128, 3], F32, tag="yt")
                    nc.vector.tensor_copy(yt[:], mix3[:, t, 384:387])
                    nc.sync.dma_start(out=y[128 * t:128 * (t + 1), :], in_=yt[:])
            # ---- phase C: GNN layers
            NLAYERS = GL if phases in ("full",) else (0 if phases.startswith(("A", "B")) else int(phases[1]))
            DO_AG = phases in ("full",) or phases.endswith("ag")
            def mm_chunk(t, l, j, pdst, start, stop):
                cj = 128 if j < 3 else 8
                pt = tps.tile([128, 128], F32, tag="tps", name="pt")
                nc.tensor.transpose(pt[0:cj, :], mix3[:, t, 128 * j:128 * j + cj], id_sb[:])
                lt = work.tile([128, 128], F32, tag="lt", name="lt")
                nc.scalar.copy(lt[0:cj, :], pt[0:cj, :])
                nc.tensor.matmul(pdst[:], lhsT=lt[0:cj, :],
                                 rhs=Wg_sb[0:cj, (4 * l + j) * GW:(4 * l + j + 1) * GW],
                                 start=start, stop=stop)

            parts = {}
            for l in range(NLAYERS):
                for t in range(TPC):
                    if l > 0:
                        g = gpool.tile([128, K * TE], TDT, tag="g16")
                        gather_split(nc, g, tabs[l], widxs[t], TE)
                        gv = g[:].rearrange("p (j e) -> p j e", e=TE)
                        u6 = gpool.tile([128, 6 * TE], TDT, tag="t6")
                        u6v = u6[:].rearrange("p (j e) -> p j e", e=TE)
                        nc.vector.tensor_add(u6v, gv[:, 0:6, :], gv[:, 6:12, :])
                        u3 = gpool.tile([128, 3 * TE], TDT, tag="t3")
                        u3v = u3[:].rearrange("p (j e) -> p j e", e=TE)
                        nc.vector.tensor_add(u3v, u6v[:, 0:3, :], u6v[:, 3:6, :])
                        u1 = gpool.tile([128, TE], TDT, tag="t1")
                        nc.vector.tensor_add(u1[:], u3[:, 0:TE], u3[:, TE:2 * TE])
                        nc.vector.tensor_add(mix3[:, t, GW:2 * GW], u1[:, 0:GW],
                                             u3[:, 2 * TE:2 * TE + GW])
                    if l == 0:
                        pmm = mmps.tile([128, GW], F32, tag="mmps")
                        for j in range(4):
                            mm_chunk(t, l, j, pmm, j == 0, j == 3)
                        zin = pmm
                    else:
                        # agg-dependent chunks only; chunks 0+3 were pre-run
                        # into parts[t] during the previous AllGather window
                        pmm = mmps.tile([128, GW], F32, tag="mmps")
                        mm_chunk(t, l, 1, pmm, True, False)
                        mm_chunk(t, l, 2, pmm, False, True)
                        zin = work.tile([128, GW], F32, tag="zadd", name="zadd")
                        nc.vector.tensor_add(zin[:], pmm[:], parts[t][:])
                    if l < GL - 1:
                        nc.scalar.activation(mix3[:, t, 0:GW], zin[:], AF.Silu)
                        nc.scalar.activation(h16buf[:, 256 * t:256 * t + GW], zin[:], AF.Silu)
                    else:
                        nc.scalar.activation(h4buf[:, GW * t:GW * (t + 1)], zin[:], AF.Silu)
                if l < GL - 1:
                    # pre-pass for next layer: agg-independent chunks, scheduled
                    # under the AllGather window
                    for t in range(TPC):
                        ph = mmps.tile([128, GW], F32, tag="mmps")
                        mm_chunk(t, l + 1, 0, ph, True, False)
                        mm_chunk(t, l + 1, 3, ph, False, True)
                        part = partp.tile([128, GW], F32, tag="part", name="part")
                        nc.scalar.copy(part[:], ph[:])
                        parts[t] = part
                    dst = ibs[l][:].rearrange("(t p) e -> p t e", p=128)
                    nc.scalar.dma_start(out=dst, in_=h16buf[:].rearrange(
                        "p (t e) -> p t e", e=256))
                if l < GL - 1 and (DO_AG or l < NLAYERS - 1):
                    nc.gpsimd.collective_compute(
                        "AllGather", ALU.bypass,
                        replica_groups=[list(range(n_cores))],
                        ins=[ibs[l][:]], outs=[tabs[l + 1][:]])

            if phases.startswith("C"):
                for t in range(TPC):
                    yt = small.tile([128, 3], F32, tag="yt")
                    src = mix3[:, t, 0:3] if NLAYERS < GL else h4buf[:, GW * t:GW * t + 3]
                    nc.vector.tensor_copy(yt[:], src)
                    nc.sync.dma_start(out=y[128 * t:128 * (t + 1), :], in_=yt[:])
            # ---- phase D: LayerNorm (gamma/beta folded into Wo) + output proj
            for t in range(TPC if phases == "full" else 0):
                h4 = h4buf[:, GW * t:GW * (t + 1)]
                ssum = small.tile([128, 1], F32, tag="ssum")
                nc.vector.reduce_sum(ssum[:], h4, axis=AX.X)
                mu = small.tile([128, 1], F32, tag="mu")
                nc.vector.tensor_scalar_mul(mu[:], ssum[:], 1.0 / GW)
                xm = work.tile([128, GW], F32, tag="xm")
                nc.vector.tensor_scalar(xm[:], h4, mu[:], None, op0=ALU.subtract)
                sq = work.tile([128, GW], F32, tag="sq")
                vsum = small.tile([128, 1], F32, tag="vsum")
                nc.scalar.activation(sq[:], xm[:], AF.Square, accum_out=vsum[:])
                sd = small.tile([128, 1], F32, tag="sd")
                nc.scalar.activation(sd[:], vsum[:], AF.Sqrt, scale=1.0 / GW, bias=eps_sb[:])
                rin = small.tile([128, 1], F32, tag="rin")
                nc.vector.reciprocal(rin[:], sd[:])
                gn = work.tile([128, GW], F32, tag="gn")
                nc.vector.tensor_scalar_mul(gn[:], xm[:], rin[:])
                # transpose gn, then y = gn @ Wo' + bo'
                pz = mmps.tile([128, GW], F32, tag="mmps")
                pt1 = tps.tile([128, 128], F32, tag="tps")
                nc.tensor.transpose(pt1[:], gn[:, 0:128], id_sb[:])
                lt1 = work.tile([128, 128], F32, tag="lt")
                nc.scalar.copy(lt1[:], pt1[:])
                pt2 = tps.tile([128, 128], F32, tag="tps")
                nc.tensor.transpose(pt2[0:64, :], gn[:, 128:192], id_sb[:])
                lt2 = work.tile([128, 128], F32, tag="lt2")
                nc.scalar.copy(lt2[0:64, :], pt2[0:64, :])
                nc.vector.memset(lt2[64:65, :], 1.0)
                nc.tensor.matmul(pz[:, 0:3], lhsT=lt1[:], rhs=Wo1_sb[:], start=True, stop=False)
                nc.tensor.matmul(pz[:, 0:3], lhsT=lt2[0:65, :], rhs=Wo2_sb[:], start=False, stop=True)
                nc.scalar.copy(ybuf[:, 3 * t:3 * (t + 1)], pz[:, 0:3])
            if phases == "full":
                dst = y[:].rearrange("(t p) e -> p t e", p=128)
                nc.sync.dma_start(out=dst, in_=ybuf[:].rearrange("p (t e) -> p t e", e=3))

    nc.finalize()
    for fn in nc.m.functions:
        for bb in fn.blocks:
            for ins_ in bb.instructions:
                if isinstance(ins_, mybir.InstCollectiveCompute):
                    ap = ins_.outs[0]
                    total = 1
                    for _s, c in ap.ap:
                        total *= c
                    assert total == N * TE, (total, ap)
                    ap.ap = [[TE, N], [1, TE]]
    if split:
        _split_waits(nc)
    return nc


# ---------------------------------------------------------------- host side

def prep_inputs(x, z, B_fourier, Wp, bp, Wg, bg, gamma, beta, Wo, bo, n_cores=NC):
    x = np.asarray(x, np.float32); z = np.asarray(z, np.float32)
    B_fourier = np.asarray(B_fourier, np.float32)
    Wp = np.asarray(Wp, np.float32); bp = np.asarray(bp, np.float32)
    Wg = np.asarray(Wg, np.float32); bg = np.asarray(bg, np.float32)
    gamma = np.asarray(gamma, np.float32); beta = np.asarray(beta, np.float32)
    Wo = np.asarray(Wo, np.float32); bo = np.asarray(bo, np.float32)

    xb = x @ B_fourier                                  # (N, 8)
    featT = np.empty((20, N), np.float32)
    featT[0:8] = np.sin(xb).T
    featT[8:16] = np.cos(xb).T
    featT[16:19] = x.T
    featT[19] = 1.0
    x_sq = np.sum(x * x, axis=1)
    xsq4 = np.concatenate([x.T, x_sq[None]], 0).astype(np.float32)

    Wp20 = np.concatenate([Wp[0:19], (bp + z @ Wp[19:])[None]], 0).astype(np.float32)

    WgA = np.zeros((128, 16 * GW), np.float32)
    for l in range(GL):
        Wg_l = np.concatenate([
            Wg[l, 0:GW],
            Wg[l, GW:2 * GW] / K,
            Wg[l, 2 * GW:2 * GW + 6],
            bg[l][None],
            np.zeros((1, GW), np.float32),
        ], 0)                                            # (392, 192)
        for j in range(4):
            cj = 128 if j < 3 else 8
            WgA[0:cj, (4 * l + j) * GW:(4 * l + j + 1) * GW] = Wg_l[128 * j:128 * j + cj]

    xs_all = np.zeros((N, 64), np.float32)
    xs_all[:, 0:3] = x
    xs_all[:, 3:6] = x * x
    xsg = xs_all.reshape(64, 128, 64).transpose(1, 0, 2).reshape(128, 64 * 64)

    WoP = (gamma[:, None] * Wo).astype(np.float32)
    boP = (beta @ Wo + bo).astype(np.float32)
    Wo1 = WoP[0:128]
    Wo2 = np.concatenate([WoP[128:192], boP[None]], 0).astype(np.float32)

    ident = np.eye(128, dtype=np.float32)

    shared = {"xsq4r": xsq4, "Wp16": Wp20.astype(np.float16),
              "feat16": featT.astype(np.float16), "WgA": WgA,
              "xst16": xsg.astype(np.float16),
              "ident": ident, "Wo1": Wo1, "Wo2": Wo2}
    in_maps = []
    for c in range(n_cores):
        rows = slice(NLOC * c, NLOC * (c + 1))
        xo = x[rows]                                     # (1024, 3)
        lhsD = np.empty((4, NLOC), np.float32)
        lhsD[0:3] = 2.0 * xo.T
        lhsD[3] = -1.0
        xonn = np.empty((128, 3 * TPC), np.float32)
        for t in range(TPC):
            xonn[:, 3 * t:3 * (t + 1)] = xo[128 * t:128 * (t + 1)]
        m = dict(shared)
        m.update({"lhs4r": lhsD,
                  "fown16": np.ascontiguousarray(featT[:, rows]).astype(np.float16)})
        m.update({"xonn": xonn})
        in_maps.append(m)
    return in_maps


_CACHE = {}


def _get_nc(n_cores=NC):
    if n_cores not in _CACHE:
        _CACHE[n_cores] = build_kernel(n_cores)
    return _CACHE[n_cores]


def kernel(x, z, B_fourier, Wp, bp, Wg, bg, gamma, beta, Wo, bo):
    from concourse.bass_utils import run_bass_kernel_spmd
    nc = _get_nc(NC)
    in_maps = prep_inputs(x, z, B_fourier, Wp, bp, Wg, bg, gamma, beta, Wo, bo, NC)
    res = run_bass_kernel_spmd(nc, in_maps, list(range(NC)))
    return np.concatenate([res.results[c]["y"] for c in range(NC)], axis=0)



# revision 23
# speedup vs baseline: 1.0179x; 1.0179x over previous
"""DisplacementNet Trainium2 kernel: kNN graph + 4 GNN message-passing layers.

Sharding: data-parallel over nodes, 1024 rows per core (8 cores). Coordinates
and weights are replicated; the kNN candidate set is the full 8192 nodes. The
per-layer feature table is all-gathered across cores between layers.

Self-contained: hardcodes all shapes; no sibling imports.
"""
import numpy as np

import concourse.bass as bass
import concourse.bacc as bacc
import concourse.mybir as mybir
import concourse.tile as tile
from concourse import library_config
from concourse.tile_rust import add_dep_helper
from concourse.vector_clock import ScopedClock

N = 8192
NC = 8
NLOC = N // NC          # 1024
TPC = NLOC // 128       # 8 tiles of 128 nodes per core
K = 12
GW = 192
GL = 4
MIXW = 392              # 390 + ones + pad
LN_EPS = 1e-3
NEG_INF = -3.0e38
F16TAB = True
F32 = mybir.dt.float32
AF = mybir.ActivationFunctionType
ALU = mybir.AluOpType
AX = mybir.AxisListType

# ---------------------------------------------------------------- tile patch

_MAXW = 1


def _patched_drain_and_barrier(self, tick_clock, wait_clock):
    nc = self.nc
    drain_inst = nc.sync.drain()
    wait_clock.add_sem_waits(
        drain_inst.ins, ScopedClock({None: tick_clock.global_clock})
    )
    si = drain_inst.ins.sync_info
    waits = list(si.on_wait or []) if si is not None else []
    if len(waits) > _MAXW:
        drain_inst.ins.sync_info = mybir.SyncInfo(
            on_wait=waits[:_MAXW], on_update=list(si.on_update or [])
        )
        rest = waits[_MAXW:]
        for i in range(0, len(rest), _MAXW):
            d2 = nc.sync.drain()
            d2.ins.sync_info = mybir.SyncInfo(on_wait=rest[i : i + _MAXW], on_update=[])
    nc.all_engine_barrier()
    assert self.sems is not None
    popped = nc._tile_sem_poison_stack.pop()
    assert popped is self._sem_poison
    nc.clear_and_free_semaphores(list(self.sems.allocated().values()))
    nc.all_engine_barrier()


tile.TileContext._drain_and_barrier = _patched_drain_and_barrier


def _split_waits(nc):
    """This walrus build allows one sync-wait per instruction; hoist extras
    onto same-engine NOPs inserted just before."""
    for f in nc.m.functions:
        for bb in f.blocks:
            out, changed = [], False
            for ins in bb.instructions:
                si = ins.sync_info
                waits = list(si.on_wait or []) if si is not None else []
                if len(waits) > 1:
                    changed = True
                    for i, w in enumerate(waits[:-1]):
                        nop = mybir.InstNoOp(
                            name=f"{ins.name}_wsplit{i}", engine=ins.engine,
                            ins=[], outs=[],
                        )
                        nop.sync_info = mybir.SyncInfo(on_wait=[w], on_update=[])
                        out.append(nop)
                    ins.sync_info = mybir.SyncInfo(
                        on_wait=[waits[-1]], on_update=list(si.on_update or [])
                    )
                out.append(ins)
            if changed:
                bb.instructions = out


# ---------------------------------------------------------------- kernel body

_GQ = [0]


def gather_split(nc, out_tile, tab, widx_t, elem):
    """dma_gather with >1024 indices fails on HW; split into 1024 + 512.
    (Only SWDGE queue 0 exists on this build, so calls serialize there.)"""
    insts = []
    for (m0, cnt) in ((0, 1024), (1024, 512)):
        q = 0
        insts.append(nc.gpsimd.dma_gather(
            out_ap=out_tile[:, (m0 // 128) * elem:((m0 + cnt) // 128) * elem]
                .rearrange("p (j e) -> p j e", e=elem),
            in_ap=tab[:],
            idxs_ap=widx_t[:, m0 // 16:(m0 + cnt) // 16].bitcast(mybir.dt.int16),
            num_idxs=cnt, num_idxs_reg=cnt, elem_size=elem, queue_num=q))
    return insts


def build_kernel(n_cores=NC, split=True, phases="full"):
    nc = bacc.Bacc(None)
    nc.gpsimd.load_library(library_config.mlp)

    F16 = mybir.dt.float16
    F32R = mybir.dt.float32r
    xsq4r = nc.dram_tensor("xsq4r", [4, N], F32, kind="ExternalInput")
    lhs4r = nc.dram_tensor("lhs4r", [4, NLOC], F32, kind="ExternalInput")
    Wp16 = nc.dram_tensor("Wp16", [20, GW], F16, kind="ExternalInput")
    WgA = nc.dram_tensor("WgA", [128, 16 * GW], F32, kind="ExternalInput")
    xst16 = nc.dram_tensor("xst16", [128, 64 * 64], F16, kind="ExternalInput")
    xonn = nc.dram_tensor("xonn", [128, 3 * TPC], F32, kind="ExternalInput")
    feat16 = nc.dram_tensor("feat16", [20, N], F16, kind="ExternalInput")
    fown16 = nc.dram_tensor("fown16", [20, NLOC], F16, kind="ExternalInput")
    ident = nc.dram_tensor("ident", [128, 128], F32, kind="ExternalInput")
    Wo1 = nc.dram_tensor("Wo1", [128, 3], F32, kind="ExternalInput")
    Wo2 = nc.dram_tensor("Wo2", [65, 3], F32, kind="ExternalInput")
    y = nc.dram_tensor("y", [NLOC, 3], F32, kind="ExternalOutput")

    with tile.TileContext(nc) as tc:
        with tc.tile_pool(name="const", bufs=1) as cpool, \
             tc.tile_pool(name="big", bufs=1) as big, \
             tc.tile_pool(name="nd", bufs=2) as ndpool, \
             tc.tile_pool(name="gath", bufs=2) as gpool, \
             tc.tile_pool(name="small", bufs=2) as small, \
             tc.tile_pool(name="work", bufs=3) as work, \
             tc.tile_pool(name="tbp", bufs=2) as tbp, \
             tc.tile_pool(name="partp", bufs=8) as partp, \
             tc.tile_pool(name="g1p", bufs=1) as g1p, \
             tc.tile_pool(name="ndps", bufs=2, space="PSUM") as ndps, \
             tc.tile_pool(name="tps", bufs=2, space="PSUM") as tps, \
             tc.tile_pool(name="mmps", bufs=2, space="PSUM") as mmps, \
             tc.tile_pool(name="dram", bufs=1, space="DRAM") as dp:

            # ---- constants into SBUF
            feat_sb = cpool.tile([20, N], F16)
            nc.sync.dma_start(out=feat_sb[:], in_=feat16[:])
            lhs_r = cpool.tile([4, NLOC], F32)
            nc.sync.dma_start(out=lhs_r[:], in_=lhs4r[:])
            rhs_r = cpool.tile([4, N], F32)
            nc.sync.dma_start(out=rhs_r[:], in_=xsq4r[:])
            Wp_sb = cpool.tile([20, GW], F16)
            nc.sync.dma_start(out=Wp_sb[:], in_=Wp16[:])
            Wg_sb = cpool.tile([128, 16 * GW], F32)
            nc.sync.dma_start(out=Wg_sb[:], in_=WgA[:])
            xonn_sb = cpool.tile([128, 3 * TPC], F32)
            nc.sync.dma_start(out=xonn_sb[:], in_=xonn[:])
            fown_sb = cpool.tile([20, NLOC], F16)
            nc.sync.dma_start(out=fown_sb[:], in_=fown16[:])
            id_sb = cpool.tile([128, 128], F32)
            nc.sync.dma_start(out=id_sb[:], in_=ident[:])
            Wo1_sb = cpool.tile([128, 3], F32)
            nc.sync.dma_start(out=Wo1_sb[:], in_=Wo1[:])
            Wo2_sb = cpool.tile([65, 3], F32)
            nc.sync.dma_start(out=Wo2_sb[:], in_=Wo2[:])
            eps_sb = cpool.tile([128, 1], F32)
            nc.vector.memset(eps_sb[:], LN_EPS)
            zpad = cpool.tile([128, 512], mybir.dt.float16)
            nc.vector.memset(zpad[:], 0.0)
            goff32 = cpool.tile([128, 64], mybir.dt.int32)
            nc.gpsimd.iota(goff32[:], pattern=[[1024, 8], [0, 8]], base=0,
                           channel_multiplier=0)

            # ---- DRAM buffers
            TDT = mybir.dt.float16 if F16TAB else F32
            TE = 256 if F16TAB else GW
            tab0 = dp.tile([N, 256], F16)
            tabs = [tab0]
            for l in range(1, GL):
                tabs.append(dp.tile([N, TE], TDT, name=f"tab{l}"))
            ibs = [dp.tile([NLOC, TE], TDT, name=f"ib{l}") for l in range(GL - 1)]
            dwidxs = [dp.tile([16, 96], mybir.dt.uint16, name=f"dw{t}") for t in range(TPC)]

            # ---- persistent SBUF state
            widxs = [big.tile([128, 96], mybir.dt.uint16, name=f"wx{t}") for t in range(TPC)]
            mixbuf = big.tile([128, TPC * MIXW], F32)
            h4buf = big.tile([128, TPC * GW], F32)
            h16buf = big.tile([128, TPC * 256], mybir.dt.float16)
            nc.vector.memset(h16buf[:], 0.0)
            ybuf = big.tile([128, TPC * 3], F32)
            var8 = big.tile([128, TPC * 3], F32)
            mix3 = mixbuf[:].rearrange("p (t c) -> p t c", c=MIXW)
            nc.vector.memset(mix3[:, :, 390:391], 1.0)
            nc.vector.memset(mix3[:, :, 391:392], 0.0)

            # ---- phase A: full hcur0 table (graph_proj), every core redundantly
            for G in range(8):
                tb = tbp.tile([128, 8 * 256], F16, tag="tb")
                tb3 = tb[:].rearrange("p (i e) -> p i e", e=256)
                # x / x^2 stats into cols 192:256 of each row (rest zeros)
                nc.sync.dma_start(out=tb3[:, :, 192:256],
                                  in_=xst16[:, 512 * G:512 * (G + 1)]
                                  .rearrange("p (i e) -> p i e", e=64))
                for i in range(8):
                    T = 8 * G + i
                    ps = mmps.tile([128, GW], F32, tag="mmps")
                    nc.tensor.matmul(ps[:], lhsT=feat_sb[:, 128 * T:128 * (T + 1)],
                                     rhs=Wp_sb[:], start=True, stop=True)
                    nc.scalar.copy(tb3[:, i, 0:GW], ps[:])
                # one batched DMA per 1024 rows: dst rows 128T+p, 512B descs
                dst = tab0[1024 * G:1024 * (G + 1), :].rearrange("(i p) e -> p i e", p=128)
                nc.scalar.dma_start(out=dst, in_=tb3)

            # own rows of hcur0 straight from per-core features (no gather)
            for t in range(TPC):
                ps = mmps.tile([128, GW], F32, tag="mmps")
                nc.tensor.matmul(ps[:], lhsT=fown_sb[:, 128 * t:128 * (t + 1)],
                                 rhs=Wp_sb[:], start=True, stop=True)
                nc.scalar.copy(mix3[:, t, 0:GW], ps[:])

            if phases == "A":
                for t in range(TPC):
                    yt = small.tile([128, 3], F32, tag="yt")
                    nc.vector.tensor_copy(yt[:], mix3[:, t, 0:3])
                    nc.sync.dma_start(out=y[128 * t:128 * (t + 1), :], in_=yt[:])
            # ---- phase B: kNN + layer-1 gather per tile
            for t in range(TPC if phases != "A" else 0):
                # per-1024-group distances into PSUM; top-8 values + local
                # indices scanned straight from PSUM (no SBUF staging), then a
                # packed-key merge: quantize value to the row's [16th, 1st]
                # range and embed the 13-bit global index in the low mantissa
                # bits, so the global top-12 needs no full-row max_index.
                cand = small.tile([128, 64], F32, tag="cand")
                cidx = small.tile([128, 64], mybir.dt.uint16, tag="cidx")
                for gq in range(8):
                    ps = ndps.tile([128, 1024], F32, tag="ndps")
                    for q in range(2):
                        nc.tensor.matmul(
                            ps[:, 512 * q:512 * (q + 1)],
                            lhsT=lhs_r[:, 128 * t:128 * (t + 1)],
                            rhs=rhs_r[:, 1024 * gq + 512 * q:1024 * gq + 512 * (q + 1)],
                            start=True, stop=True)
                    nc.vector.max(cand[:, 8 * gq:8 * (gq + 1)], ps[:])
                    nc.vector.max_index(cidx[:, 8 * gq:8 * (gq + 1)],
                                        cand[:, 8 * gq:8 * (gq + 1)], ps[:])
                v1 = small.tile([128, 8], F32, tag="v1")
                nc.vector.max(v1[:], cand[:])
                cand2 = small.tile([128, 64], F32, tag="cand2")
                nc.vector.match_replace(cand2[:], v1[:], cand[:], NEG_INF)
                v2 = small.tile([128, 8], F32, tag="v2")
                nc.vector.max(v2[:], cand2[:])
                rng = small.tile([128, 1], F32, tag="rng")
                nc.vector.tensor_sub(rng[:], v1[:, 0:1], v2[:, 7:8])
                rngc = small.tile([128, 1], F32, tag="rngc")
                nc.vector.tensor_scalar_max(rngc[:], rng[:], 1e-20)
                rcp = small.tile([128, 1], F32, tag="rcp")
                nc.vector.reciprocal(rcp[:], rngc[:])
                rcp2 = small.tile([128, 1], F32, tag="rcp2")
                nc.vector.tensor_scalar_mul(rcp2[:], rcp[:], 1048576.0)
                t0 = small.tile([128, 64], F32, tag="t0")
                nc.vector.tensor_scalar(t0[:], cand[:], v2[:, 7:8], rcp2[:],
                                        op0=ALU.subtract, op1=ALU.mult)
                t1 = small.tile([128, 64], F32, tag="t1")
                nc.vector.tensor_scalar(t1[:], t0[:], 0.0, 1048576.0,
                                        op0=ALU.max, op1=ALU.add)
                tq = small.tile([128, 64], mybir.dt.int32, tag="tq")
                nc.vector.tensor_scalar(tq[:], t1[:].bitcast(mybir.dt.int32),
                                        -8192, None, op0=ALU.bitwise_and)
                gidx = small.tile([128, 64], mybir.dt.int32, tag="gidx")
                nc.vector.tensor_copy(gidx[:], cidx[:])
                gidx2 = small.tile([128, 64], mybir.dt.int32, tag="gidx2")
                nc.vector.tensor_add(gidx2[:], gidx[:], goff32[:])
                packedi = small.tile([128, 64], mybir.dt.int32, tag="packedi")
                nc.vector.tensor_tensor(packedi[:], gidx2[:], tq[:],
                                        op=ALU.bitwise_or)
                kp1 = small.tile([128, 8], F32, tag="kp1")
                nc.vector.max(kp1[:], packedi[:].bitcast(F32))
                pk2 = small.tile([128, 64], F32, tag="pk2")
                nc.vector.match_replace(pk2[:], kp1[:], packedi[:].bitcast(F32),
                                        NEG_INF)
                kp2 = small.tile([128, 8], F32, tag="kp2")
                nc.vector.max(kp2[:], pk2[:])
                knp = small.tile([128, 12], mybir.dt.int32, tag="knp")
                nc.vector.tensor_scalar(knp[:, 0:7],
                                        kp1[:, 1:8].bitcast(mybir.dt.int32),
                                        8191, None, op0=ALU.bitwise_and)
                nc.vector.tensor_scalar(knp[:, 7:12],
                                        kp2[:, 0:5].bitcast(mybir.dt.int32),
                                        8191, None, op0=ALU.bitwise_and)
                # wrapped idx build through DRAM: m = j*128+p at [m%16, 96t + m//16]
                kn = small.tile([128, 12], mybir.dt.uint16, tag="kn")
                nc.vector.tensor_copy(kn[:], knp[:])
                d1 = dwidxs[t][:].rearrange("q (j phi) -> phi q j", phi=8)
                w1 = nc.sync.dma_start(out=d1, in_=kn[:])
                # replicate the 16-partition wrapped list to all 8 groups by doubling
                reps = []
                r = nc.sync.dma_start(out=widxs[t][0:16, :], in_=dwidxs[t][:])
                add_dep_helper(r.ins, w1.ins, True, "rep after build")
                reps.append(r)
                for (src_n, eng) in ((16, nc.scalar), (32, nc.sync), (64, nc.scalar)):
                    r = eng.dma_start(out=widxs[t][src_n:2 * src_n, :],
                                      in_=widxs[t][0:src_n, :])
                    reps.append(r)

                if phases == "BK":
                    yt = small.tile([128, 3], F32, tag="yt")
                    nc.vector.tensor_copy(yt[:], kn[:, 0:3])
                    nc.sync.dma_start(out=y[128 * t:128 * (t + 1), :], in_=yt[:])
                    continue
                # layer-1 feature gather; x/x2 stats ride along in cols 192:198
                # (SWDGE gather tops out at 1024 indices -> split 1024 + 512)
                g = g1p.tile([128, K * 256], F16, tag="g")
                gi = gather_split(nc, g, tab0, widxs[t], 256)
                if phases.startswith("BG2"):
                    yt = small.tile([128, 3], F32, tag="yt")
                    nc.vector.tensor_copy(yt[:], g[:, 0:3])
                    nc.sync.dma_start(out=y[128 * t:128 * (t + 1), :], in_=yt[:])
                    for r in reps:
                        for gg in gi:
                            add_dep_helper(gg.ins, r.ins, True, "gather after widx")
                    continue
                if phases == "BG":
                    gv = g[:].rearrange("p (j e) -> p j e", e=256)[:, :, 0:GW]
                    nc.vector.reduce_sum(mix3[:, t, GW:2 * GW],
                                   gv.rearrange("p j e -> p e j"), axis=AX.X)
                    yt = small.tile([128, 3], F32, tag="yt")
                    nc.vector.tensor_copy(yt[:], mix3[:, t, GW:GW + 3])
                    nc.sync.dma_start(out=y[128 * t:128 * (t + 1), :], in_=yt[:])
                    for r in reps:
                        for gg in gi:
                            add_dep_helper(gg.ins, r.ins, True, "gather after widx")
                    continue
                for r in reps:
                    for gg in gi:
                        add_dep_helper(gg.ins, r.ins, True, "gather after widx")

                # agg-sum for layer 1 via f16 tree-adds (2x DVE mode); the
                # x/x^2 stat columns (192:198) ride along in the same adds
                gv = g[:].rearrange("p (j e) -> p j e", e=256)
                t6 = gpool.tile([128, 6 * 256], F16, tag="t6")
                t6v = t6[:].rearrange("p (j e) -> p j e", e=256)
                nc.vector.tensor_add(t6v, gv[:, 0:6, :], gv[:, 6:12, :])
                t3 = gpool.tile([128, 3 * 256], F16, tag="t3")
                t3v = t3[:].rearrange("p (j e) -> p j e", e=256)
                nc.vector.tensor_add(t3v, t6v[:, 0:3, :], t6v[:, 3:6, :])
                t1 = gpool.tile([128, 256], F16, tag="t1")
                nc.vector.tensor_add(t1[:], t3[:, 0:256], t3[:, 256:512])
                nc.vector.tensor_add(mix3[:, t, GW:2 * GW], t1[:, 0:GW],
                                     t3[:, 512:512 + GW])
                # rel stats (layer-invariant): mean + population std via E[x2]-E[x]^2
                s6 = small.tile([128, 6], F32, tag="s6")
                nc.vector.tensor_add(s6[:], t1[:, GW:GW + 6],
                                     t3[:, 512 + GW:512 + GW + 6])
                m6 = small.tile([128, 6], F32, tag="m6")
                nc.vector.tensor_scalar_mul(m6[:], s6[:], 1.0 / K)
                nc.vector.tensor_sub(mix3[:, t, 384:387], m6[:, 0:3], xonn_sb[:, 3 * t:3 * (t + 1)])
                msq = small.tile([128, 3], F32, tag="msq")
                nc.vector.tensor_mul(msq[:], m6[:, 0:3], m6[:, 0:3])
                var3 = small.tile([128, 3], F32, tag="var3")
                nc.vector.tensor_sub(var3[:], m6[:, 3:6], msq[:])
                var3r = small.tile([128, 3], F32, tag="var3r")
                nc.vector.tensor_scalar_max(var3r[:], var3[:], 0.0)
                nc.scalar.activation(mix3[:, t, 387:390], var3r[:], AF.Sqrt)

            if phases == "B":
                for t in range(TPC):
                    yt = small.tile([128, 3], F32, tag="yt")
                    nc.vector.tensor_copy(yt[:], mix3[:, t, 384:387])
                    nc.sync.dma_start(out=y[128 * t:128 * (t + 1), :], in_=yt[:])
            # ---- phase C: GNN layers
            NLAYERS = GL if phases in ("full",) else (0 if phases.startswith(("A", "B")) else int(phases[1]))
            DO_AG = phases in ("full",) or phases.endswith("ag")
            def mm_chunk(t, l, j, pdst, start, stop):
                cj = 128 if j < 3 else 8
                pt = tps.tile([128, 128], F32, tag="tps", name="pt")
                nc.tensor.transpose(pt[0:cj, :], mix3[:, t, 128 * j:128 * j + cj], id_sb[:])
                lt = work.tile([128, 128], F32, tag="lt", name="lt")
                nc.scalar.copy(lt[0:cj, :], pt[0:cj, :])
                nc.tensor.matmul(pdst[:], lhsT=lt[0:cj, :],
                                 rhs=Wg_sb[0:cj, (4 * l + j) * GW:(4 * l + j + 1) * GW],
                                 start=start, stop=stop)

            parts = {}
            for l in range(NLAYERS):
                for t in range(TPC):
                    if l > 0:
                        g = gpool.tile([128, K * TE], TDT, tag="g16")
                        gather_split(nc, g, tabs[l], widxs[t], TE)
                        gv = g[:].rearrange("p (j e) -> p j e", e=TE)
                        u6 = gpool.tile([128, 6 * TE], TDT, tag="t6")
                        u6v = u6[:].rearrange("p (j e) -> p j e", e=TE)
                        nc.vector.tensor_add(u6v, gv[:, 0:6, :], gv[:, 6:12, :])
                        u3 = gpool.tile([128, 3 * TE], TDT, tag="t3")
                        u3v = u3[:].rearrange("p (j e) -> p j e", e=TE)
                        nc.vector.tensor_add(u3v, u6v[:, 0:3, :], u6v[:, 3:6, :])
                        u1 = gpool.tile([128, TE], TDT, tag="t1")
                        nc.vector.tensor_add(u1[:], u3[:, 0:TE], u3[:, TE:2 * TE])
                        nc.vector.tensor_add(mix3[:, t, GW:2 * GW], u1[:, 0:GW],
                                             u3[:, 2 * TE:2 * TE + GW])
                    if l == 0:
                        pmm = mmps.tile([128, GW], F32, tag="mmps")
                        for j in range(4):
                            mm_chunk(t, l, j, pmm, j == 0, j == 3)
                        zin = pmm
                    else:
                        # agg-dependent chunks only; chunks 0+3 were pre-run
                        # into parts[t] during the previous AllGather window
                        pmm = mmps.tile([128, GW], F32, tag="mmps")
                        mm_chunk(t, l, 1, pmm, True, False)
                        mm_chunk(t, l, 2, pmm, False, True)
                        zin = work.tile([128, GW], F32, tag="zadd", name="zadd")
                        nc.vector.tensor_add(zin[:], pmm[:], parts[t][:])
                    if l < GL - 1:
                        nc.scalar.activation(mix3[:, t, 0:GW], zin[:], AF.Silu)
                        nc.scalar.activation(h16buf[:, 256 * t:256 * t + GW], zin[:], AF.Silu)
                    else:
                        nc.scalar.activation(h4buf[:, GW * t:GW * (t + 1)], zin[:], AF.Silu)
                if l < GL - 1:
                    # pre-pass for next layer: agg-independent chunks, scheduled
                    # under the AllGather window
                    for t in range(TPC):
                        ph = mmps.tile([128, GW], F32, tag="mmps")
                        mm_chunk(t, l + 1, 0, ph, True, False)
                        mm_chunk(t, l + 1, 3, ph, False, True)
                        part = partp.tile([128, GW], F32, tag="part", name="part")
                        nc.scalar.copy(part[:], ph[:])
                        parts[t] = part
                    dst = ibs[l][:].rearrange("(t p) e -> p t e", p=128)
                    nc.scalar.dma_start(out=dst, in_=h16buf[:].rearrange(
                        "p (t e) -> p t e", e=256))
                if l < GL - 1 and (DO_AG or l < NLAYERS - 1):
                    nc.gpsimd.collective_compute(
                        "AllGather", ALU.bypass,
                        replica_groups=[list(range(n_cores))],
                        ins=[ibs[l][:]], outs=[tabs[l + 1][:]])

            if phases.startswith("C"):
                for t in range(TPC):
                    yt = small.tile([128, 3], F32, tag="yt")
                    src = mix3[:, t, 0:3] if NLAYERS < GL else h4buf[:, GW * t:GW * t + 3]
                    nc.vector.tensor_copy(yt[:], src)
                    nc.sync.dma_start(out=y[128 * t:128 * (t + 1), :], in_=yt[:])
            # ---- phase D: LayerNorm (gamma/beta folded into Wo) + output proj
            for t in range(TPC if phases == "full" else 0):
                h4 = h4buf[:, GW * t:GW * (t + 1)]
                ssum = small.tile([128, 1], F32, tag="ssum")
                nc.vector.reduce_sum(ssum[:], h4, axis=AX.X)
                mu = small.tile([128, 1], F32, tag="mu")
                nc.vector.tensor_scalar_mul(mu[:], ssum[:], 1.0 / GW)
                xm = work.tile([128, GW], F32, tag="xm")
                nc.vector.tensor_scalar(xm[:], h4, mu[:], None, op0=ALU.subtract)
                sq = work.tile([128, GW], F32, tag="sq")
                vsum = small.tile([128, 1], F32, tag="vsum")
                nc.scalar.activation(sq[:], xm[:], AF.Square, accum_out=vsum[:])
                sd = small.tile([128, 1], F32, tag="sd")
                nc.scalar.activation(sd[:], vsum[:], AF.Sqrt, scale=1.0 / GW, bias=eps_sb[:])
                rin = small.tile([128, 1], F32, tag="rin")
                nc.vector.reciprocal(rin[:], sd[:])
                gn = work.tile([128, GW], F32, tag="gn")
                nc.vector.tensor_scalar_mul(gn[:], xm[:], rin[:])
                # transpose gn, then y = gn @ Wo' + bo'
                pz = mmps.tile([128, GW], F32, tag="mmps")
                pt1 = tps.tile([128, 128], F32, tag="tps")
                nc.tensor.transpose(pt1[:], gn[:, 0:128], id_sb[:])
                lt1 = work.tile([128, 128], F32, tag="lt")
                nc.scalar.copy(lt1[:], pt1[:])
                pt2 = tps.tile([128, 128], F32, tag="tps")
                nc.tensor.transpose(pt2[0:64, :], gn[:, 128:192], id_sb[:])
                lt2 = work.tile([128, 128], F32, tag="lt2")
                nc.scalar.copy(lt2[0:64, :], pt2[0:64, :])
                nc.vector.memset(lt2[64:65, :], 1.0)
                nc.tensor.matmul(pz[:, 0:3], lhsT=lt1[:], rhs=Wo1_sb[:], start=True, stop=False)
                nc.tensor.matmul(pz[:, 0:3], lhsT=lt2[0:65, :], rhs=Wo2_sb[:], start=False, stop=True)
                nc.scalar.copy(ybuf[:, 3 * t:3 * (t + 1)], pz[:, 0:3])
            if phases == "full":
                dst = y[:].rearrange("(t p) e -> p t e", p=128)
                nc.sync.dma_start(out=dst, in_=ybuf[:].rearrange("p (t e) -> p t e", e=3))

    nc.finalize()
    for fn in nc.m.functions:
        for bb in fn.blocks:
            for ins_ in bb.instructions:
                if isinstance(ins_, mybir.InstCollectiveCompute):
                    ap = ins_.outs[0]
                    total = 1
                    for _s, c in ap.ap:
                        total *= c
                    assert total == N * TE, (total, ap)
                    ap.ap = [[TE, N], [1, TE]]
    if split:
        _split_waits(nc)
    return nc


# ---------------------------------------------------------------- host side

def prep_inputs(x, z, B_fourier, Wp, bp, Wg, bg, gamma, beta, Wo, bo, n_cores=NC):
    x = np.asarray(x, np.float32); z = np.asarray(z, np.float32)
    B_fourier = np.asarray(B_fourier, np.float32)
    Wp = np.asarray(Wp, np.float32); bp = np.asarray(bp, np.float32)
    Wg = np.asarray(Wg, np.float32); bg = np.asarray(bg, np.float32)
    gamma = np.asarray(gamma, np.float32); beta = np.asarray(beta, np.float32)
    Wo = np.asarray(Wo, np.float32); bo = np.asarray(bo, np.float32)

    xb = x @ B_fourier                                  # (N, 8)
    featT = np.empty((20, N), np.float32)
    featT[0:8] = np.sin(xb).T
    featT[8:16] = np.cos(xb).T
    featT[16:19] = x.T
    featT[19] = 1.0
    x_sq = np.sum(x * x, axis=1)
    xsq4 = np.concatenate([x.T, x_sq[None]], 0).astype(np.float32)

    Wp20 = np.concatenate([Wp[0:19], (bp + z @ Wp[19:])[None]], 0).astype(np.float32)

    WgA = np.zeros((128, 16 * GW), np.float32)
    for l in range(GL):
        Wg_l = np.concatenate([
            Wg[l, 0:GW],
            Wg[l, GW:2 * GW] / K,
            Wg[l, 2 * GW:2 * GW + 6],
            bg[l][None],
            np.zeros((1, GW), np.float32),
        ], 0)                                            # (392, 192)
        for j in range(4):
            cj = 128 if j < 3 else 8
            WgA[0:cj, (4 * l + j) * GW:(4 * l + j + 1) * GW] = Wg_l[128 * j:128 * j + cj]

    xs_all = np.zeros((N, 64), np.float32)
    xs_all[:, 0:3] = x
    xs_all[:, 3:6] = x * x
    xsg = xs_all.reshape(64, 128, 64).transpose(1, 0, 2).reshape(128, 64 * 64)

    WoP = (gamma[:, None] * Wo).astype(np.float32)
    boP = (beta @ Wo + bo).astype(np.float32)
    Wo1 = WoP[0:128]
    Wo2 = np.concatenate([WoP[128:192], boP[None]], 0).astype(np.float32)

    ident = np.eye(128, dtype=np.float32)

    shared = {"xsq4r": xsq4, "Wp16": Wp20.astype(np.float16),
              "feat16": featT.astype(np.float16), "WgA": WgA,
              "xst16": xsg.astype(np.float16),
              "ident": ident, "Wo1": Wo1, "Wo2": Wo2}
    in_maps = []
    for c in range(n_cores):
        rows = slice(NLOC * c, NLOC * (c + 1))
        xo = x[rows]                                     # (1024, 3)
        lhsD = np.empty((4, NLOC), np.float32)
        lhsD[0:3] = 2.0 * xo.T
        lhsD[3] = -1.0
        xonn = np.empty((128, 3 * TPC), np.float32)
        for t in range(TPC):
            xonn[:, 3 * t:3 * (t + 1)] = xo[128 * t:128 * (t + 1)]
        m = dict(shared)
        m.update({"lhs4r": lhsD,
                  "fown16": np.ascontiguousarray(featT[:, rows]).astype(np.float16)})
        m.update({"xonn": xonn})
        in_maps.append(m)
    return in_maps


_CACHE = {}


def _get_nc(n_cores=NC):
    if n_cores not in _CACHE:
        _CACHE[n_cores] = build_kernel(n_cores)
    return _CACHE[n_cores]


def kernel(x, z, B_fourier, Wp, bp, Wg, bg, gamma, beta, Wo, bo):
    from concourse.bass_utils import run_bass_kernel_spmd
    nc = _get_nc(NC)
    in_maps = prep_inputs(x, z, B_fourier, Wp, bp, Wg, bg, gamma, beta, Wo, bo, NC)
    res = run_bass_kernel_spmd(nc, in_maps, list(range(NC)))
    return np.concatenate([res.results[c]["y"] for c in range(NC)], axis=0)

